# revision 56
# baseline (speedup 1.0000x reference)
"""C2Q attention kernel for 8 TRN2 NeuronCores, ragged-aware.

Math (per batch):
    u      = o_q @ W.T + b                       [Tq, H]
    score  = (o_c @ u.T) / sqrt(H)               [Tc, Tq]
    prob   = softmax_j(score masked at j>=q_len) [Tc, Tq]
    out    = (prob * (i < c_len)) @ o_q          [Tc, H]

Ragged scheme: lengths are in [Tq/2, Tq] x [Tc/2, Tc]. All 8 cores run one
SPMD program with 4 batch slots; slot s is compiled with budgets
Bq_s = max(q_len) / Bc_s = max(c_len) over the 8 batches assigned to that
slot (one per core). Batches are assigned to slots by a cost sort + local
search so budgets hug the actual lengths. Every matmul free dim and tile
count is sized to the slot budget:
    Linear  : 64 matmuls, N = Bq
    score   : 8 * jt_n matmuls, N = Bc, stationary M partial on last j tile
    context : 2 * it_n * jt_n matmuls, K partial on last j tile
    denom   : jt_n matmuls N = Bc; it_n K=1 transposes for 1/d
Device layout (everything K-on-partitions, no on-chip transposes):
    u computed as [o, j]  (lhsT = W.T[h, o] tile, rhs = o_qT[h, j])
    score computed TRANSPOSED e=[j, i] (lhsT = u[o, j-block], rhs = o_cT[o, i])
    exp via ACT with per-partition bias qb[j] in {0, -60000}: masked -> 0
    denominator d[1, i] = ones[j,1].T @ e  (matmul partition-reduce)
    1/d transposed to columns via K=1 matmuls, folded into context eviction
    context [i, h] = e[j, i-block].T @ o_q[j, h]
c_len row masking is applied host-side (rows >= c_len are never copied out).
"""

import os
import sys

import numpy as np

if "/opt/trn_rl_repo" not in sys.path:
    sys.path.insert(0, "/opt/trn_rl_repo")

B, Tc, Tq, H = 32, 512, 512, 1024
N_CORES = 8
N_SLOTS = B // N_CORES  # 4
KT = H // 128  # contraction tiles over h (8)
OT = H // 128  # linear-output tiles over o (8)
JT = Tq // 128  # max question-token tiles (4)
HB = H // 512  # free-dim blocks for context matmul (2)
SCALE = 1.0 / 32.0  # 1/sqrt(H)
WTW = H + 16  # wt slab width (pad)
QTW = Tq + 8  # oqT slab width (pad); qb rides at cols [Bq, Bq+jt_n)
CTW = Tc + 8  # ocT slab width (pad); ones column at col Bc of k=0 slab


def _ceil_div(a, b):
    return -(-a // b)


def _slot_cost(bq, bc):
    """Per-batch PE-time model (in 2.4GHz cycles) for budget (bq, bc)."""
    jt = max(1, _ceil_div(bq, 128))
    it = max(1, _ceil_div(bc, 128))
    lin = 64 * (bq + 30)
    score = 8 * jt * (bc + 30)
    ctx = 2 * it * jt * (512 + 30)
    den = bc + 16
    return lin + score + ctx + den


def _plan(q_lengths, c_lengths):
    """Assign batches to (core, slot) minimizing total slot-budget cost.

    Returns perm[slot][core] -> global batch idx, budgets[slot] = (Bq, Bc).
    """
    ql = np.clip(np.asarray(q_lengths, dtype=np.int64), 1, Tq)
    cl = np.clip(np.asarray(c_lengths, dtype=np.int64), 1, Tc)
    cost = np.array([_slot_cost(q, c) for q, c in zip(ql, cl)])
    order = np.argsort(cost, kind="stable")
    slots = [list(order[s * N_CORES : (s + 1) * N_CORES]) for s in range(N_SLOTS)]

    def total(sl):
        t = 0
        for members in sl:
            bq = max(int(ql[b]) for b in members)
            bc = max(int(cl[b]) for b in members)
            t += _slot_cost(bq, bc)
        return t

    best = total(slots)
    improved = True
    while improved:
        improved = False
        for s1 in range(N_SLOTS):
            for s2 in range(s1 + 1, N_SLOTS):
                for i in range(N_CORES):
                    for j in range(N_CORES):
                        a, b_ = slots[s1][i], slots[s2][j]
                        slots[s1][i], slots[s2][j] = b_, a
                        t = total(slots)
                        if t < best:
                            best = t
                            improved = True
                        else:
                            slots[s1][i], slots[s2][j] = a, b_
    budgets = []
    for members in slots:
        bq = max(int(ql[b]) for b in members)
        bc = max(int(cl[b]) for b in members)
        budgets.append((bq, bc))
    # order slots: second-cheapest first (small ramp DMA), cheapest LAST
    # (smallest drain tail: final evict + out-DMA scale with the last
    # slot's tail i-tile)
    idx = sorted(range(N_SLOTS), key=lambda s: _slot_cost(*budgets[s]))
    idx = idx[1:] + idx[:1]
    slots = [slots[s] for s in idx]
    budgets = [budgets[s] for s in idx]
    return slots, budgets


def _build_program(budgets):
    import concourse.bacc as bacc
    import concourse.mybir as mybir
    import concourse.tile as tile

    f32 = mybir.dt.float32
    f16 = mybir.dt.float16

    nc = bacc.Bacc("TRN2", debug=False)

    oqT_d = nc.declare_dram_parameter("oqT", [N_SLOTS, 128, KT, QTW], f16, isOutput=False)
    ocT_d = nc.declare_dram_parameter("ocT", [N_SLOTS, 128, KT, CTW], f16, isOutput=False)
    oqN_d = nc.declare_dram_parameter("oqN", [N_SLOTS, 128, JT, H], f16, isOutput=False)
    wt_d = nc.declare_dram_parameter("wt", [128, KT, WTW], f16, isOutput=False)
    bias_d = nc.declare_dram_parameter("biasP", [128, OT], f32, isOutput=False)
    out_d = nc.declare_dram_parameter("out", [N_SLOTS, Tc, H], f16, isOutput=True)
    # softmax denominators, one row per slot; the 1/d scaling happens on the
    # host (it is a per-output-row scalar), which removes the K=1 transpose
    # matmuls + reciprocals + the r-dependency from the eviction path
    d_d = nc.declare_dram_parameter("dout", [N_SLOTS, 1, Tc], f32, isOutput=True)

    with tile.TileContext(nc) as tc:
        with (
            tc.tile_pool(name="const", bufs=1) as cpool,
            tc.tile_pool(name="inp", bufs=1) as ipool,
            tc.tile_pool(name="work", bufs=1) as wpool,
            tc.tile_pool(name="outp", bufs=4) as opool,
            tc.tile_pool(name="ps_u", bufs=2, space="PSUM") as ps_u,
            tc.tile_pool(name="ps_s", bufs=2, space="PSUM") as ps_s,
            tc.tile_pool(name="ps_c", bufs=4, space="PSUM") as ps_c,
        ):
            ones_s = cpool.tile([1, 1], f32)
            nc.vector.memset(ones_s, 1.0)

            # HAM warm-up: the PE sits idle for ~5us between its init barrier
            # and the first DMA-fed matmul, which leaves the clock gate at
            # 4/8 (1.2 GHz) for the first ~3.4us of real work. Tiny K=1
            # matmuls don't register as busy (too low duty); a short burst
            # of full-size N=512 matmuls on a scratch tile does, and it
            # completes before the first input slabs land. Once warm, the
            # <3.4us idle until the real work does not re-throttle.
            junk = cpool.tile([128, 512], f16, tag="junk", name="junk")
            nc.vector.memset(junk, 0.0)
            jps = ps_c.tile([128, 512], f32, tag="cps", name="warm_ps")
            for w in range(10):
                nc.tensor.matmul(
                    jps,
                    junk[:, 0:128],
                    junk,
                    start=True,
                    stop=True,
                )

            wt = cpool.tile([128, KT, WTW], f16, tag="wt", name="wt")
            biasP = cpool.tile([128, OT], f32)

            # --- per-slot geometry + input tiles, all DMAs issued upfront ---
            geo = []
            for s, (Bq, Bc) in enumerate(budgets):
                jt_n = max(1, _ceil_div(Bq, 128))
                it_n = max(1, _ceil_div(Bc, 128))
                QW = Bq + 8
                CW = Bc + 8
                oqT = ipool.tile([128, KT, QW], f16, tag=f"oqT{s}")
                ocT = ipool.tile([128, KT, CW], f16, tag=f"ocT{s}")
                oqN = ipool.tile([128, jt_n, H], f16, tag=f"oqN{s}")
                geo.append((Bq, Bc, jt_n, it_n, oqT, ocT, oqN))

            # DMA order: slot-0 wt/oqT split in k-ranges (0, 1:4, 4:8) so the
            # ramp-critical Linear can start after ~360KB; bias rides third
            # (only needed at the first Linear eviction). Everything else
            # merged, one trigger per slab (the Sync trigger stream is serial
            # at ~0.6us per dma_start).
            # DMA trigger descriptor-generation is the ramp bottleneck
            # (~0.6us per ~0.25MB, serial per queue). Split the streams:
            # wt/bias on the Sync queue, oqT/ocT/oqN on the (otherwise
            # idle) GpSimd queue -- the two generate in parallel.
            Bq0, Bc0, jt0, _, oqT0, ocT0, oqN0 = geo[0]
            for k in range(KT):
                nc.sync.dma_start(out=wt[:, k, :], in_=wt_d[:, k, :])
                nc.gpsimd.dma_start(
                    out=oqT0[:, k, :], in_=oqT_d[0, :, k, : Bq0 + 8]
                )
                if k == 0:
                    nc.sync.dma_start(out=biasP, in_=bias_d[:, :])
                if k >= 4:
                    # slot-0 score needs ocT right after Linear
                    ko = 2 * (k - 4)
                    nc.gpsimd.dma_start(
                        out=ocT0[:, ko : ko + 2, :],
                        in_=ocT_d[0, :, ko : ko + 2, : Bc0 + 8],
                    )
            nc.gpsimd.dma_start(out=oqN0, in_=oqN_d[0, :, :jt0, :])

            def issue_inputs(s):
                """Input DMAs for slot s, split in k-halves so no single
                trigger blocks its queue for more than ~2.5us (descriptor
                generation scales with bytes); oqT on Sync, ocT/oqN on
                GpSimd."""
                Bq, Bc, jt_n, it_n, oqT, ocT, oqN = geo[s]
                for lo, hi in ((0, 4), (4, 8)):
                    nc.sync.dma_start(
                        out=oqT[:, lo:hi, :], in_=oqT_d[s, :, lo:hi, : Bq + 8]
                    )
                for lo, hi in ((0, 4), (4, 8)):
                    nc.gpsimd.dma_start(
                        out=ocT[:, lo:hi, :], in_=ocT_d[s, :, lo:hi, : Bc + 8]
                    )
                nc.gpsimd.dma_start(out=oqN, in_=oqN_d[s, :, :jt_n, :])

            # slot 1's inputs go out up front; slot s+2's are issued after
            # slot s's output triggers (see the bottom of the slot loop) so
            # bulky input descriptor-generation never sits ahead of the
            # eviction-critical output triggers in the serial Sync queue.
            if N_SLOTS > 1:
                issue_inputs(1)

            for s in range(N_SLOTS):
                Bq, Bc, jt_n, it_n, oqT, ocT, oqN = geo[s]

                qb = oqT[:, KT - 1, Bq : Bq + jt_n]
                ones = ocT[:, 0, Bc : Bc + 1]

                # ---- Linear: u[o, j] = W @ o_q.T + b ----
                u = wpool.tile([128, OT, Bq], f16, tag=f"u{s}")
                if s == 0:
                    # k-outer with 8 open PSUM o-groups (banks borrowed from
                    # every pool -- nothing else is in PSUM yet): each wt/oqT
                    # k-slab is consumed the moment its DMA lands, so the
                    # ramp is gated by the DMA trigger stream, not by
                    # o-group serialization.
                    pools8 = [ps_u, ps_u, ps_s, ps_s, ps_c, ps_c, ps_c, ps_c]
                    tags8 = ["ups", "ups", "sps", "sps", "cps", "cps", "cps", "cps"]
                    upss = [
                        pools8[o].tile(
                            [128, Bq], f32, tag=tags8[o], name=f"ups0_{o}"
                        )
                        for o in range(OT)
                    ]
                    for k in range(KT):
                        for o in range(OT):
                            nc.tensor.matmul(
                                upss[o],
                                wt[:, k, o * 128 : (o + 1) * 128],
                                oqT[:, k, :Bq],
                                start=(k == 0),
                                stop=(k == KT - 1),
                            )
                    for o in range(OT):
                        nc.vector.tensor_scalar(
                            out=u[:, o, :],
                            in0=upss[o],
                            scalar1=biasP[:, o : o + 1],
                            scalar2=None,
                            op0=mybir.AluOpType.add,
                        )
                else:
                    for o in range(OT):
                        ups = ps_u.tile([128, Bq], f32, tag="ups")
                        for k in range(KT):
                            nc.tensor.matmul(
                                ups,
                                wt[:, k, o * 128 : (o + 1) * 128],
                                oqT[:, k, :Bq],
                                start=(k == 0),
                                stop=(k == KT - 1),
                            )
                        nc.vector.tensor_scalar(
                            out=u[:, o, :],
                            in0=ups,
                            scalar1=biasP[:, o : o + 1],
                            scalar2=None,
                            op0=mybir.AluOpType.add,
                        )

                # ---- score_T + exp: e[j, i] = exp((u.T @ o_cT)/32 + qb[j]).
                # The e tiles are pre-summed on DVE (esum) so the denominator
                # d[1, i] needs a single partition-reduce matmul instead of
                # jt_n of them. Rows [kj, 128) of esum hold the full-tile
                # partial sums only, which is exactly right: the last tile's
                # missing rows don't exist as tokens.
                e_tiles = []
                e_rows = []
                esum = wpool.tile([128, Bc], f16, tag=f"esum{s}")
                for jt in range(jt_n):
                    mj = min(128, Bq - jt * 128)
                    sps = ps_s.tile([128, Bc], f32, tag="sps")
                    for o in range(OT):
                        nc.tensor.matmul(
                            sps[0:mj, :],
                            u[:, o, jt * 128 : jt * 128 + mj],
                            ocT[:, o, :Bc],
                            start=(o == 0),
                            stop=(o == OT - 1),
                        )
                    e = wpool.tile([128, Bc], f16, tag=f"e{s}_{jt}")
                    nc.scalar.activation(
                        out=e[0:mj, :],
                        in_=sps[0:mj, :],
                        func=mybir.ActivationFunctionType.Exp,
                        bias=qb[0:mj, jt : jt + 1],
                        scale=SCALE,
                    )
                    e_tiles.append(e)
                    e_rows.append(mj)
                    if jt == 1:
                        nc.vector.tensor_tensor(
                            out=esum[0 : e_rows[1], :],
                            in0=e_tiles[0][0 : e_rows[1], :],
                            in1=e_tiles[1][0 : e_rows[1], :],
                            op=mybir.AluOpType.add,
                        )
                        if e_rows[1] < 128:
                            nc.vector.tensor_copy(
                                out=esum[e_rows[1] : 128, :],
                                in_=e_tiles[0][e_rows[1] : 128, :],
                            )
                    elif jt >= 2:
                        nc.vector.tensor_tensor(
                            out=esum[0:mj, :],
                            in0=esum[0:mj, :],
                            in1=e[0:mj, :],
                            op=mybir.AluOpType.add,
                        )
                osb_tiles = {}

                def ctx_group_pair(it, mi):
                    """Both hb halves for one i-tile. Even i-tiles take PSUM
                    from ps_c, odd from ps_u (idle during the ctx phase), so
                    two pairs can be in flight without a 9th bank."""
                    if it not in osb_tiles:
                        osb_tiles[it] = opool.tile(
                            [128, H], f16, tag="osb", name=f"osb{it}_{s}"
                        )
                    cps = [
                        ps_c.tile([128, 512], f32, tag="cps", name=f"cps{it}{hb}_{s}")
                        for hb in range(HB)
                    ]
                    for jt in range(jt_n):
                        kj = e_rows[jt]
                        for hb in range(HB):
                            inst = nc.tensor.matmul(
                                cps[hb][0:mi, :],
                                e_tiles[jt][0:kj, it * 128 : it * 128 + mi],
                                oqN[0:kj, jt, hb * 512 : (hb + 1) * 512],
                                start=(jt == 0),
                                stop=(jt == jt_n - 1),
                            )
                            if hb > 0:
                                inst.ins.ldweights = False
                    return cps

                def ctx_evict(it, hb, mi, cps):
                    osb = osb_tiles[it]
                    nc.vector.tensor_copy(
                        out=osb[0:mi, hb * 512 : (hb + 1) * 512],
                        in_=cps[0:mi, :],
                    )
                    if s == N_SLOTS - 1:
                        # drain path: trigger each half as soon as it is
                        # evicted so the final transfers start ~1.5us earlier
                        nc.sync.dma_start(
                            out=out_d[
                                s, it * 128 : it * 128 + mi, hb * 512 : (hb + 1) * 512
                            ],
                            in_=osb[0:mi, hb * 512 : (hb + 1) * 512],
                        )
                    elif hb == HB - 1:
                        nc.sync.dma_start(
                            out=out_d[s, it * 128 : it * 128 + mi, :],
                            in_=osb[0:mi, :],
                        )

                def mi_of(it):
                    return min(128, Bc - it * 128)

                # two ctx pairs in flight before the d-chain: the d-matmul
                # waits on DVE's esum and the 1/d transposes wait on the dsb
                # copy -- pair 1's matmuls keep the PE fed through both.
                pend = {0: ctx_group_pair(0, mi_of(0))}
                if it_n > 1:
                    pend[1] = ctx_group_pair(1, mi_of(1))
                dps = ps_s.tile([1, Bc], f32, tag="sps", name=f"dps_{s}")
                dsrc = esum if jt_n > 1 else e_tiles[0]
                drows = 128 if jt_n > 1 else e_rows[0]
                nc.tensor.matmul(
                    dps,
                    ones[0:drows, :],
                    dsrc[0:drows, :],
                    start=True,
                    stop=True,
                )
                dsb = wpool.tile([1, Bc], f32, tag=f"dsb{s}")
                nc.vector.tensor_copy(out=dsb, in_=dps)
                nc.sync.dma_start(out=d_d[s, 0:1, :Bc], in_=dsb[0:1, :])

                for it in range(it_n):
                    for hb in range(HB):
                        ctx_evict(it, hb, mi_of(it), pend[it][hb])
                    nxt = it + 2
                    if nxt < it_n:
                        # keep one pair in flight ahead of the evictions
                        pend[nxt] = ctx_group_pair(nxt, mi_of(nxt))

                if s + 2 < N_SLOTS:
                    issue_inputs(s + 2)

    nc.compile()
    return nc


def _host_inputs(o_c, o_q, W, b, q_lengths, slots, budgets):
    """Build the per-core input maps (host-side sharding + re-layout)."""
    NEG16 = np.float16(-60000.0)  # exp(x - 60000) == 0 exactly in fp32
    # wt[p, k, col] = W.T[k*128+p, col] (partition-major slab)
    wt_host = np.zeros((128, KT, WTW), np.float16)
    wt_host[:, :, :H] = W.T.reshape(KT, 128, H).transpose(1, 0, 2)
    bias_host = np.ascontiguousarray(b.reshape(OT, 128).T)  # [128, o_tile] f32
    o_q16 = o_q.astype(np.float16)
    o_c16 = o_c.astype(np.float16)
    in_maps = []
    for c in range(N_CORES):
        oqT = np.zeros((N_SLOTS, 128, KT, QTW), np.float16)
        ocT = np.zeros((N_SLOTS, 128, KT, CTW), np.float16)
        oqN = np.zeros((N_SLOTS, 128, JT, H), np.float16)
        for s in range(N_SLOTS):
            g = slots[s][c]
            Bq, Bc = budgets[s]
            jt_n = max(1, _ceil_div(Bq, 128))
            # oqT[p, k, j] = o_q[j, k*128+p]
            oqT[s, :, :, :Tq] = o_q16[g].T.reshape(KT, 128, Tq).transpose(1, 0, 2)
            ocT[s, :, :, :Tc] = o_c16[g].T.reshape(KT, 128, Tc).transpose(1, 0, 2)
            # oqN[p, j, h] = o_q[j*128+p, h]
            oqN[s] = o_q16[g].reshape(JT, 128, H).transpose(1, 0, 2)
            ocT[s, :, 0, Bc] = 1.0  # ones column for the denominator matmul
            ql = int(q_lengths[g])
            jidx = np.arange(jt_n)[None, :] * 128 + np.arange(128)[:, None]
            oqT[s, :, KT - 1, Bq : Bq + jt_n] = np.where(
                jidx < ql, np.float16(0.0), NEG16
            )
        in_maps.append(
            {"oqT": oqT, "ocT": ocT, "oqN": oqN, "wt": wt_host, "biasP": bias_host}
        )
    return in_maps


def kernel(**inputs) -> np.ndarray:
    o_c = np.asarray(inputs["o_c"], dtype=np.float32)
    o_q = np.asarray(inputs["o_q"], dtype=np.float32)
    W = np.asarray(inputs["W"], dtype=np.float32)
    b = np.asarray(inputs["b"], dtype=np.float32)
    q_lengths = np.asarray(inputs["q_lengths"]).astype(np.int64)
    c_lengths = np.asarray(inputs["c_lengths"]).astype(np.int64)

    from concourse.bass_utils import run_bass_kernel_spmd

    slots, budgets = _plan(q_lengths, c_lengths)
    in_maps = _host_inputs(o_c, o_q, W, b, q_lengths, slots, budgets)
    nc = _build_program(budgets)

    trace = bool(int(os.environ.get("KERNEL_TRACE", "0")))
    res = run_bass_kernel_spmd(
        nc, in_maps, core_ids=list(range(N_CORES)), trace=trace
    )
    if trace:
        kernel.last_results = res

    out = np.zeros((B, Tc, H), dtype=np.float32)
    for c in range(N_CORES):
        dev = res.results[c]["out"]
        dvec = np.asarray(res.results[c]["dout"], dtype=np.float32)
        for s in range(N_SLOTS):
            g = slots[s][c]
            cl = int(c_lengths[g])
            out[g, :cl] = dev[s, :cl].astype(np.float32) / dvec[s, 0, :cl, None]
    return out


# revision 58
# speedup vs baseline: 1.0359x; 1.0359x over previous
"""C2Q attention kernel for 8 TRN2 NeuronCores, ragged-aware.

Math (per batch):
    u      = o_q @ W.T + b                       [Tq, H]
    score  = (o_c @ u.T) / sqrt(H)               [Tc, Tq]
    prob   = softmax_j(score masked at j>=q_len) [Tc, Tq]
    out    = (prob * (i < c_len)) @ o_q          [Tc, H]

Ragged scheme: lengths are in [Tq/2, Tq] x [Tc/2, Tc]. All 8 cores run one
SPMD program with 4 batch slots; slot s is compiled with budgets
Bq_s = max(q_len) / Bc_s = max(c_len) over the 8 batches assigned to that
slot (one per core). Batches are assigned to slots by a cost sort + local
search so budgets hug the actual lengths. Every matmul free dim and tile
count is sized to the slot budget:
    Linear  : 64 matmuls, N = Bq
    score   : 8 * jt_n matmuls, N = Bc, stationary M partial on last j tile
    context : 2 * it_n * jt_n matmuls, K partial on last j tile
    denom   : jt_n matmuls N = Bc; it_n K=1 transposes for 1/d
Device layout (everything K-on-partitions, no on-chip transposes):
    u computed as [o, j]  (lhsT = W.T[h, o] tile, rhs = o_qT[h, j])
    score computed TRANSPOSED e=[j, i] (lhsT = u[o, j-block], rhs = o_cT[o, i])
    exp via ACT with per-partition bias qb[j] in {0, -60000}: masked -> 0
    denominator d[1, i] = ones[j,1].T @ e  (matmul partition-reduce)
    1/d transposed to columns via K=1 matmuls, folded into context eviction
    context [i, h] = e[j, i-block].T @ o_q[j, h]
c_len row masking is applied host-side (rows >= c_len are never copied out).
"""

import os
import sys

import numpy as np

if "/opt/trn_rl_repo" not in sys.path:
    sys.path.insert(0, "/opt/trn_rl_repo")

B, Tc, Tq, H = 32, 512, 512, 1024
N_CORES = 8
N_SLOTS = B // N_CORES  # 4
KT = H // 128  # contraction tiles over h (8)
OT = H // 128  # linear-output tiles over o (8)
JT = Tq // 128  # max question-token tiles (4)
HB = H // 512  # free-dim blocks for context matmul (2)
SCALE = 1.0 / 32.0  # 1/sqrt(H)
WTW = H + 16  # wt slab width (pad)
QTW = Tq + 8  # oqT slab width (pad); qb rides at cols [Bq, Bq+jt_n)
CTW = Tc + 8  # ocT slab width (pad); ones column at col Bc of k=0 slab


def _ceil_div(a, b):
    return -(-a // b)


def _slot_cost(bq, bc):
    """Per-batch PE-time model (in 2.4GHz cycles) for budget (bq, bc)."""
    jt = max(1, _ceil_div(bq, 128))
    it = max(1, _ceil_div(bc, 128))
    lin = 64 * (bq + 30)
    score = 8 * jt * (bc + 30)
    ctx = 2 * it * jt * (512 + 30)
    den = bc + 16
    return lin + score + ctx + den


def _plan(q_lengths, c_lengths):
    """Assign batches to (core, slot) minimizing total slot-budget cost.

    Returns perm[slot][core] -> global batch idx, budgets[slot] = (Bq, Bc).
    """
    ql = np.clip(np.asarray(q_lengths, dtype=np.int64), 1, Tq)
    cl = np.clip(np.asarray(c_lengths, dtype=np.int64), 1, Tc)
    cost = np.array([_slot_cost(q, c) for q, c in zip(ql, cl)])
    order = np.argsort(cost, kind="stable")
    slots = [list(order[s * N_CORES : (s + 1) * N_CORES]) for s in range(N_SLOTS)]

    def total(sl):
        t = 0
        for members in sl:
            bq = max(int(ql[b]) for b in members)
            bc = max(int(cl[b]) for b in members)
            t += _slot_cost(bq, bc)
        return t

    best = total(slots)
    improved = True
    while improved:
        improved = False
        for s1 in range(N_SLOTS):
            for s2 in range(s1 + 1, N_SLOTS):
                for i in range(N_CORES):
                    for j in range(N_CORES):
                        a, b_ = slots[s1][i], slots[s2][j]
                        slots[s1][i], slots[s2][j] = b_, a
                        t = total(slots)
                        if t < best:
                            best = t
                            improved = True
                        else:
                            slots[s1][i], slots[s2][j] = a, b_
    budgets = []
    for members in slots:
        bq = max(int(ql[b]) for b in members)
        bc = max(int(cl[b]) for b in members)
        budgets.append((bq, bc))
    # order slots: second-cheapest first (small ramp DMA), cheapest LAST
    # (smallest drain tail: final evict + out-DMA scale with the last
    # slot's tail i-tile)
    idx = sorted(range(N_SLOTS), key=lambda s: _slot_cost(*budgets[s]))
    idx = idx[1:] + idx[:1]
    slots = [slots[s] for s in idx]
    budgets = [budgets[s] for s in idx]
    return slots, budgets


def _build_program(budgets):
    import concourse.bacc as bacc
    import concourse.mybir as mybir
    import concourse.tile as tile

    f32 = mybir.dt.float32
    f16 = mybir.dt.float16

    nc = bacc.Bacc("TRN2", debug=False)

    oqT_d = nc.declare_dram_parameter("oqT", [N_SLOTS, 128, KT, QTW], f16, isOutput=False)
    ocT_d = nc.declare_dram_parameter("ocT", [N_SLOTS, 128, KT, CTW], f16, isOutput=False)
    oqN_d = nc.declare_dram_parameter("oqN", [N_SLOTS, 128, JT, H], f16, isOutput=False)
    wt_d = nc.declare_dram_parameter("wt", [128, KT, WTW], f16, isOutput=False)
    bias_d = nc.declare_dram_parameter("biasP", [128, OT], f32, isOutput=False)
    out_d = nc.declare_dram_parameter("out", [N_SLOTS, Tc, H], f16, isOutput=True)
    # softmax denominators, one row per slot; the 1/d scaling happens on the
    # host (it is a per-output-row scalar), which removes the K=1 transpose
    # matmuls + reciprocals + the r-dependency from the eviction path
    d_d = nc.declare_dram_parameter("dout", [N_SLOTS, 1, Tc], f32, isOutput=True)

    with tile.TileContext(nc) as tc:
        with (
            tc.tile_pool(name="const", bufs=1) as cpool,
            tc.tile_pool(name="inp", bufs=1) as ipool,
            tc.tile_pool(name="work", bufs=1) as wpool,
            tc.tile_pool(name="outp", bufs=4) as opool,
            tc.tile_pool(name="ps_u", bufs=2, space="PSUM") as ps_u,
            tc.tile_pool(name="ps_s", bufs=2, space="PSUM") as ps_s,
            tc.tile_pool(name="ps_c", bufs=4, space="PSUM") as ps_c,
        ):
            ones_s = cpool.tile([1, 1], f32)
            nc.vector.memset(ones_s, 1.0)

            # HAM warm-up: the PE sits idle for ~5us between its init barrier
            # and the first DMA-fed matmul, which leaves the clock gate at
            # 4/8 (1.2 GHz) for the first ~3.4us of real work. Tiny K=1
            # matmuls don't register as busy (too low duty); a short burst
            # of full-size N=512 matmuls on a scratch tile does, and it
            # completes before the first input slabs land. Once warm, the
            # <3.4us idle until the real work does not re-throttle.
            junk = cpool.tile([128, 512], f16, tag="junk", name="junk")
            nc.vector.memset(junk, 0.0)
            jps = ps_c.tile([128, 512], f32, tag="cps", name="warm_ps")
            for w in range(10):
                nc.tensor.matmul(
                    jps,
                    junk[:, 0:128],
                    junk,
                    start=True,
                    stop=True,
                )

            wt = cpool.tile([128, KT, WTW], f16, tag="wt", name="wt")
            biasP = cpool.tile([128, OT], f32)

            # --- per-slot geometry + input tiles, all DMAs issued upfront ---
            geo = []
            for s, (Bq, Bc) in enumerate(budgets):
                jt_n = max(1, _ceil_div(Bq, 128))
                it_n = max(1, _ceil_div(Bc, 128))
                QW = Bq + 8
                CW = Bc + 8
                oqT = ipool.tile([128, KT, QW], f16, tag=f"oqT{s}")
                ocT = ipool.tile([128, KT, CW], f16, tag=f"ocT{s}")
                oqN = ipool.tile([128, jt_n, H], f16, tag=f"oqN{s}")
                geo.append((Bq, Bc, jt_n, it_n, oqT, ocT, oqN))

            # DMA order: slot-0 wt/oqT split in k-ranges (0, 1:4, 4:8) so the
            # ramp-critical Linear can start after ~360KB; bias rides third
            # (only needed at the first Linear eviction). Everything else
            # merged, one trigger per slab (the Sync trigger stream is serial
            # at ~0.6us per dma_start).
            # DMA trigger descriptor-generation is the ramp bottleneck
            # (~0.6us per ~0.25MB, serial per queue). Split the streams:
            # wt/bias on the Sync queue, oqT/ocT/oqN on the (otherwise
            # idle) GpSimd queue -- the two generate in parallel.
            Bq0, Bc0, jt0, _, oqT0, ocT0, oqN0 = geo[0]
            for k in range(KT):
                nc.sync.dma_start(out=wt[:, k, :], in_=wt_d[:, k, :])
                nc.gpsimd.dma_start(
                    out=oqT0[:, k, :], in_=oqT_d[0, :, k, : Bq0 + 8]
                )
                if k == 0:
                    nc.sync.dma_start(out=biasP, in_=bias_d[:, :])
                if k >= 4:
                    # slot-0 score needs ocT right after Linear
                    ko = 2 * (k - 4)
                    nc.gpsimd.dma_start(
                        out=ocT0[:, ko : ko + 2, :],
                        in_=ocT_d[0, :, ko : ko + 2, : Bc0 + 8],
                    )
            for j in range(jt0):
                # per-tile transfers parallelize across DMA engines
                # (a single large DMA moves at only ~66GB/s)
                nc.gpsimd.dma_start(
                    out=oqN0[:, j : j + 1, :], in_=oqN_d[0, :, j : j + 1, :]
                )

            def issue_inputs(s):
                """Input DMAs for slot s, split in k-halves so no single
                trigger blocks its queue for more than ~2.5us (descriptor
                generation scales with bytes); oqT on Sync, ocT/oqN on
                GpSimd."""
                Bq, Bc, jt_n, it_n, oqT, ocT, oqN = geo[s]
                for lo, hi in ((0, 4), (4, 8)):
                    nc.sync.dma_start(
                        out=oqT[:, lo:hi, :], in_=oqT_d[s, :, lo:hi, : Bq + 8]
                    )
                for lo, hi in ((0, 4), (4, 8)):
                    nc.gpsimd.dma_start(
                        out=ocT[:, lo:hi, :], in_=ocT_d[s, :, lo:hi, : Bc + 8]
                    )
                for j in range(jt_n):
                    nc.gpsimd.dma_start(
                        out=oqN[:, j : j + 1, :], in_=oqN_d[s, :, j : j + 1, :]
                    )

            # slot 1's inputs go out up front; slot s+2's are issued after
            # slot s's output triggers (see the bottom of the slot loop) so
            # bulky input descriptor-generation never sits ahead of the
            # eviction-critical output triggers in the serial Sync queue.
            if N_SLOTS > 1:
                issue_inputs(1)

            for s in range(N_SLOTS):
                Bq, Bc, jt_n, it_n, oqT, ocT, oqN = geo[s]

                qb = oqT[:, KT - 1, Bq : Bq + jt_n]
                ones = ocT[:, 0, Bc : Bc + 1]

                # ---- Linear: u[o, j] = W @ o_q.T + b ----
                u = wpool.tile([128, OT, Bq], f16, tag=f"u{s}")
                if s == 0:
                    # k-outer with 8 open PSUM o-groups (banks borrowed from
                    # every pool -- nothing else is in PSUM yet): each wt/oqT
                    # k-slab is consumed the moment its DMA lands, so the
                    # ramp is gated by the DMA trigger stream, not by
                    # o-group serialization.
                    pools8 = [ps_u, ps_u, ps_s, ps_s, ps_c, ps_c, ps_c, ps_c]
                    tags8 = ["ups", "ups", "sps", "sps", "cps", "cps", "cps", "cps"]
                    upss = [
                        pools8[o].tile(
                            [128, Bq], f32, tag=tags8[o], name=f"ups0_{o}"
                        )
                        for o in range(OT)
                    ]
                    for k in range(KT):
                        for o in range(OT):
                            nc.tensor.matmul(
                                upss[o],
                                wt[:, k, o * 128 : (o + 1) * 128],
                                oqT[:, k, :Bq],
                                start=(k == 0),
                                stop=(k == KT - 1),
                            )
                    for o in range(OT):
                        nc.vector.tensor_scalar(
                            out=u[:, o, :],
                            in0=upss[o],
                            scalar1=biasP[:, o : o + 1],
                            scalar2=None,
                            op0=mybir.AluOpType.add,
                        )
                else:
                    for o in range(OT):
                        ups = ps_u.tile([128, Bq], f32, tag="ups")
                        for k in range(KT):
                            nc.tensor.matmul(
                                ups,
                                wt[:, k, o * 128 : (o + 1) * 128],
                                oqT[:, k, :Bq],
                                start=(k == 0),
                                stop=(k == KT - 1),
                            )
                        nc.vector.tensor_scalar(
                            out=u[:, o, :],
                            in0=ups,
                            scalar1=biasP[:, o : o + 1],
                            scalar2=None,
                            op0=mybir.AluOpType.add,
                        )

                # ---- score_T + exp: e[j, i] = exp((u.T @ o_cT)/32 + qb[j]).
                # The e tiles are pre-summed on DVE (esum) so the denominator
                # d[1, i] needs a single partition-reduce matmul instead of
                # jt_n of them. Rows [kj, 128) of esum hold the full-tile
                # partial sums only, which is exactly right: the last tile's
                # missing rows don't exist as tokens.
                e_tiles = []
                e_rows = []
                esum = wpool.tile([128, Bc], f16, tag=f"esum{s}")
                for jt in range(jt_n):
                    mj = min(128, Bq - jt * 128)
                    sps = ps_s.tile([128, Bc], f32, tag="sps")
                    for o in range(OT):
                        nc.tensor.matmul(
                            sps[0:mj, :],
                            u[:, o, jt * 128 : jt * 128 + mj],
                            ocT[:, o, :Bc],
                            start=(o == 0),
                            stop=(o == OT - 1),
                        )
                    e = wpool.tile([128, Bc], f16, tag=f"e{s}_{jt}")
                    nc.scalar.activation(
                        out=e[0:mj, :],
                        in_=sps[0:mj, :],
                        func=mybir.ActivationFunctionType.Exp,
                        bias=qb[0:mj, jt : jt + 1],
                        scale=SCALE,
                    )
                    e_tiles.append(e)
                    e_rows.append(mj)
                    if jt == 1:
                        nc.vector.tensor_tensor(
                            out=esum[0 : e_rows[1], :],
                            in0=e_tiles[0][0 : e_rows[1], :],
                            in1=e_tiles[1][0 : e_rows[1], :],
                            op=mybir.AluOpType.add,
                        )
                        if e_rows[1] < 128:
                            nc.vector.tensor_copy(
                                out=esum[e_rows[1] : 128, :],
                                in_=e_tiles[0][e_rows[1] : 128, :],
                            )
                    elif jt >= 2:
                        nc.vector.tensor_tensor(
                            out=esum[0:mj, :],
                            in0=esum[0:mj, :],
                            in1=e[0:mj, :],
                            op=mybir.AluOpType.add,
                        )
                osb_tiles = {}

                def ctx_group_pair(it, mi):
                    """Both hb halves for one i-tile. Even i-tiles take PSUM
                    from ps_c, odd from ps_u (idle during the ctx phase), so
                    two pairs can be in flight without a 9th bank."""
                    if it not in osb_tiles:
                        osb_tiles[it] = opool.tile(
                            [128, H], f16, tag="osb", name=f"osb{it}_{s}"
                        )
                    cps = [
                        ps_c.tile([128, 512], f32, tag="cps", name=f"cps{it}{hb}_{s}")
                        for hb in range(HB)
                    ]
                    for jt in range(jt_n):
                        kj = e_rows[jt]
                        for hb in range(HB):
                            inst = nc.tensor.matmul(
                                cps[hb][0:mi, :],
                                e_tiles[jt][0:kj, it * 128 : it * 128 + mi],
                                oqN[0:kj, jt, hb * 512 : (hb + 1) * 512],
                                start=(jt == 0),
                                stop=(jt == jt_n - 1),
                            )
                            if hb > 0:
                                inst.ins.ldweights = False
                    return cps

                def ctx_evict(it, hb, mi, cps):
                    osb = osb_tiles[it]
                    nc.vector.tensor_copy(
                        out=osb[0:mi, hb * 512 : (hb + 1) * 512],
                        in_=cps[0:mi, :],
                    )
                    if s == N_SLOTS - 1:
                        # drain path: trigger each half as soon as it is
                        # evicted so the final transfers start ~1.5us earlier
                        nc.sync.dma_start(
                            out=out_d[
                                s, it * 128 : it * 128 + mi, hb * 512 : (hb + 1) * 512
                            ],
                            in_=osb[0:mi, hb * 512 : (hb + 1) * 512],
                        )
                    elif hb == HB - 1:
                        nc.sync.dma_start(
                            out=out_d[s, it * 128 : it * 128 + mi, :],
                            in_=osb[0:mi, :],
                        )

                def mi_of(it):
                    return min(128, Bc - it * 128)

                # two ctx pairs in flight before the d-chain: the d-matmul
                # waits on DVE's esum and the 1/d transposes wait on the dsb
                # copy -- pair 1's matmuls keep the PE fed through both.
                pend = {0: ctx_group_pair(0, mi_of(0))}
                if it_n > 1:
                    pend[1] = ctx_group_pair(1, mi_of(1))
                dps = ps_s.tile([1, Bc], f32, tag="sps", name=f"dps_{s}")
                dsrc = esum if jt_n > 1 else e_tiles[0]
                drows = 128 if jt_n > 1 else e_rows[0]
                nc.tensor.matmul(
                    dps,
                    ones[0:drows, :],
                    dsrc[0:drows, :],
                    start=True,
                    stop=True,
                )
                dsb = wpool.tile([1, Bc], f32, tag=f"dsb{s}")
                nc.vector.tensor_copy(out=dsb, in_=dps)
                nc.sync.dma_start(out=d_d[s, 0:1, :Bc], in_=dsb[0:1, :])

                for it in range(it_n):
                    for hb in range(HB):
                        ctx_evict(it, hb, mi_of(it), pend[it][hb])
                    nxt = it + 2
                    if nxt < it_n:
                        # keep one pair in flight ahead of the evictions
                        pend[nxt] = ctx_group_pair(nxt, mi_of(nxt))

                if s + 2 < N_SLOTS:
                    issue_inputs(s + 2)

    nc.compile()
    return nc


def _host_inputs(o_c, o_q, W, b, q_lengths, slots, budgets):
    """Build the per-core input maps (host-side sharding + re-layout)."""
    NEG16 = np.float16(-60000.0)  # exp(x - 60000) == 0 exactly in fp32
    # wt[p, k, col] = W.T[k*128+p, col] (partition-major slab)
    wt_host = np.zeros((128, KT, WTW), np.float16)
    wt_host[:, :, :H] = W.T.reshape(KT, 128, H).transpose(1, 0, 2)
    bias_host = np.ascontiguousarray(b.reshape(OT, 128).T)  # [128, o_tile] f32
    o_q16 = o_q.astype(np.float16)
    o_c16 = o_c.astype(np.float16)
    in_maps = []
    for c in range(N_CORES):
        oqT = np.zeros((N_SLOTS, 128, KT, QTW), np.float16)
        ocT = np.zeros((N_SLOTS, 128, KT, CTW), np.float16)
        oqN = np.zeros((N_SLOTS, 128, JT, H), np.float16)
        for s in range(N_SLOTS):
            g = slots[s][c]
            Bq, Bc = budgets[s]
            jt_n = max(1, _ceil_div(Bq, 128))
            # oqT[p, k, j] = o_q[j, k*128+p]
            oqT[s, :, :, :Tq] = o_q16[g].T.reshape(KT, 128, Tq).transpose(1, 0, 2)
            ocT[s, :, :, :Tc] = o_c16[g].T.reshape(KT, 128, Tc).transpose(1, 0, 2)
            # oqN[p, j, h] = o_q[j*128+p, h]
            oqN[s] = o_q16[g].reshape(JT, 128, H).transpose(1, 0, 2)
            ocT[s, :, 0, Bc] = 1.0  # ones column for the denominator matmul
            ql = int(q_lengths[g])
            jidx = np.arange(jt_n)[None, :] * 128 + np.arange(128)[:, None]
            oqT[s, :, KT - 1, Bq : Bq + jt_n] = np.where(
                jidx < ql, np.float16(0.0), NEG16
            )
        in_maps.append(
            {"oqT": oqT, "ocT": ocT, "oqN": oqN, "wt": wt_host, "biasP": bias_host}
        )
    return in_maps


def kernel(**inputs) -> np.ndarray:
    o_c = np.asarray(inputs["o_c"], dtype=np.float32)
    o_q = np.asarray(inputs["o_q"], dtype=np.float32)
    W = np.asarray(inputs["W"], dtype=np.float32)
    b = np.asarray(inputs["b"], dtype=np.float32)
    q_lengths = np.asarray(inputs["q_lengths"]).astype(np.int64)
    c_lengths = np.asarray(inputs["c_lengths"]).astype(np.int64)

    from concourse.bass_utils import run_bass_kernel_spmd

    slots, budgets = _plan(q_lengths, c_lengths)
    in_maps = _host_inputs(o_c, o_q, W, b, q_lengths, slots, budgets)
    nc = _build_program(budgets)

    trace = bool(int(os.environ.get("KERNEL_TRACE", "0")))
    res = run_bass_kernel_spmd(
        nc, in_maps, core_ids=list(range(N_CORES)), trace=trace
    )
    if trace:
        kernel.last_results = res

    out = np.zeros((B, Tc, H), dtype=np.float32)
    for c in range(N_CORES):
        dev = res.results[c]["out"]
        dvec = np.asarray(res.results[c]["dout"], dtype=np.float32)
        for s in range(N_SLOTS):
            g = slots[s][c]
            cl = int(c_lengths[g])
            out[g, :cl] = dev[s, :cl].astype(np.float32) / dvec[s, 0, :cl, None]
    return out


# revision 59
# speedup vs baseline: 1.0742x; 1.0370x over previous
"""C2Q attention kernel for 8 TRN2 NeuronCores, ragged-aware.

Math (per batch):
    u      = o_q @ W.T + b                       [Tq, H]
    score  = (o_c @ u.T) / sqrt(H)               [Tc, Tq]
    prob   = softmax_j(score masked at j>=q_len) [Tc, Tq]
    out    = (prob * (i < c_len)) @ o_q          [Tc, H]

Ragged scheme: lengths are in [Tq/2, Tq] x [Tc/2, Tc]. All 8 cores run one
SPMD program with 4 batch slots; slot s is compiled with budgets
Bq_s = max(q_len) / Bc_s = max(c_len) over the 8 batches assigned to that
slot (one per core). Batches are assigned to slots by a cost sort + local
search so budgets hug the actual lengths. Every matmul free dim and tile
count is sized to the slot budget:
    Linear  : 64 matmuls, N = Bq
    score   : 8 * jt_n matmuls, N = Bc, stationary M partial on last j tile
    context : 2 * it_n * jt_n matmuls, K partial on last j tile
    denom   : jt_n matmuls N = Bc; it_n K=1 transposes for 1/d
Device layout (everything K-on-partitions, no on-chip transposes):
    u computed as [o, j]  (lhsT = W.T[h, o] tile, rhs = o_qT[h, j])
    score computed TRANSPOSED e=[j, i] (lhsT = u[o, j-block], rhs = o_cT[o, i])
    exp via ACT with per-partition bias qb[j] in {0, -60000}: masked -> 0
    denominator d[1, i] = ones[j,1].T @ e  (matmul partition-reduce)
    1/d transposed to columns via K=1 matmuls, folded into context eviction
    context [i, h] = e[j, i-block].T @ o_q[j, h]
c_len row masking is applied host-side (rows >= c_len are never copied out).
"""

import os
import sys

import numpy as np

if "/opt/trn_rl_repo" not in sys.path:
    sys.path.insert(0, "/opt/trn_rl_repo")

B, Tc, Tq, H = 32, 512, 512, 1024
N_CORES = 8
N_SLOTS = B // N_CORES  # 4
KT = H // 128  # contraction tiles over h (8)
OT = H // 128  # linear-output tiles over o (8)
JT = Tq // 128  # max question-token tiles (4)
HB = H // 512  # free-dim blocks for context matmul (2)
SCALE = 1.0 / 32.0  # 1/sqrt(H)
WTW = H + 16  # wt slab width (pad)
QTW = Tq + 8  # oqT slab width (pad); qb rides at cols [Bq, Bq+jt_n)
CTW = Tc + 8  # ocT slab width (pad); ones column at col Bc of k=0 slab


def _ceil_div(a, b):
    return -(-a // b)


def _slot_cost(bq, bc):
    """Per-batch PE-time model (in 2.4GHz cycles) for budget (bq, bc)."""
    jt = max(1, _ceil_div(bq, 128))
    it = max(1, _ceil_div(bc, 128))
    lin = 64 * (bq + 30)
    score = 8 * jt * (bc + 30)
    ctx = 2 * it * jt * (512 + 30)
    den = bc + 16
    return lin + score + ctx + den


def _plan(q_lengths, c_lengths):
    """Assign batches to (core, slot) minimizing total slot-budget cost.

    Returns perm[slot][core] -> global batch idx, budgets[slot] = (Bq, Bc).
    """
    ql = np.clip(np.asarray(q_lengths, dtype=np.int64), 1, Tq)
    cl = np.clip(np.asarray(c_lengths, dtype=np.int64), 1, Tc)
    cost = np.array([_slot_cost(q, c) for q, c in zip(ql, cl)])
    order = np.argsort(cost, kind="stable")
    slots = [list(order[s * N_CORES : (s + 1) * N_CORES]) for s in range(N_SLOTS)]

    def total(sl):
        t = 0
        for members in sl:
            bq = max(int(ql[b]) for b in members)
            bc = max(int(cl[b]) for b in members)
            t += _slot_cost(bq, bc)
        return t

    best = total(slots)
    improved = True
    while improved:
        improved = False
        for s1 in range(N_SLOTS):
            for s2 in range(s1 + 1, N_SLOTS):
                for i in range(N_CORES):
                    for j in range(N_CORES):
                        a, b_ = slots[s1][i], slots[s2][j]
                        slots[s1][i], slots[s2][j] = b_, a
                        t = total(slots)
                        if t < best:
                            best = t
                            improved = True
                        else:
                            slots[s1][i], slots[s2][j] = a, b_
    budgets = []
    for members in slots:
        bq = max(int(ql[b]) for b in members)
        bc = max(int(cl[b]) for b in members)
        budgets.append((bq, bc))
    # order slots: second-cheapest first (small ramp DMA), cheapest LAST
    # (smallest drain tail: final evict + out-DMA scale with the last
    # slot's tail i-tile)
    idx = sorted(range(N_SLOTS), key=lambda s: _slot_cost(*budgets[s]))
    idx = idx[1:] + idx[:1]
    slots = [slots[s] for s in idx]
    budgets = [budgets[s] for s in idx]
    return slots, budgets


def _build_program(budgets):
    import concourse.bacc as bacc
    import concourse.mybir as mybir
    import concourse.tile as tile

    f32 = mybir.dt.float32
    f16 = mybir.dt.float16

    nc = bacc.Bacc("TRN2", debug=False)

    oqT_d = nc.declare_dram_parameter("oqT", [N_SLOTS, 128, KT, QTW], f16, isOutput=False)
    ocT_d = nc.declare_dram_parameter("ocT", [N_SLOTS, 128, KT, CTW], f16, isOutput=False)
    oqN_d = nc.declare_dram_parameter("oqN", [N_SLOTS, 128, JT, H], f16, isOutput=False)
    wt_d = nc.declare_dram_parameter("wt", [128, KT, WTW], f16, isOutput=False)
    bias_d = nc.declare_dram_parameter("biasP", [128, OT], f32, isOutput=False)
    out_d = nc.declare_dram_parameter("out", [N_SLOTS, Tc, H], f16, isOutput=True)
    # softmax denominators, one row per slot; the 1/d scaling happens on the
    # host (it is a per-output-row scalar), which removes the K=1 transpose
    # matmuls + reciprocals + the r-dependency from the eviction path
    d_d = nc.declare_dram_parameter("dout", [N_SLOTS, 1, Tc], f32, isOutput=True)

    with tile.TileContext(nc) as tc:
        with (
            tc.tile_pool(name="const", bufs=1) as cpool,
            tc.tile_pool(name="inp", bufs=1) as ipool,
            tc.tile_pool(name="work", bufs=1) as wpool,
            tc.tile_pool(name="outp", bufs=4) as opool,
            tc.tile_pool(name="ps_u", bufs=2, space="PSUM") as ps_u,
            tc.tile_pool(name="ps_s", bufs=2, space="PSUM") as ps_s,
            tc.tile_pool(name="ps_c", bufs=4, space="PSUM") as ps_c,
        ):
            ones_s = cpool.tile([1, 1], f32)
            nc.vector.memset(ones_s, 1.0)

            # HAM warm-up: the PE sits idle for ~5us between its init barrier
            # and the first DMA-fed matmul, which leaves the clock gate at
            # 4/8 (1.2 GHz) for the first ~3.4us of real work. Tiny K=1
            # matmuls don't register as busy (too low duty); a short burst
            # of full-size N=512 matmuls on a scratch tile does, and it
            # completes before the first input slabs land. Once warm, the
            # <3.4us idle until the real work does not re-throttle.
            junk = cpool.tile([128, 512], f16, tag="junk", name="junk")
            nc.vector.memset(junk, 0.0)
            jps = ps_c.tile([128, 512], f32, tag="cps", name="warm_ps")
            for w in range(10):
                nc.tensor.matmul(
                    jps,
                    junk[:, 0:128],
                    junk,
                    start=True,
                    stop=True,
                )

            wt = cpool.tile([128, KT, WTW], f16, tag="wt", name="wt")
            biasP = cpool.tile([128, OT], f32)

            # --- per-slot geometry + input tiles, all DMAs issued upfront ---
            geo = []
            for s, (Bq, Bc) in enumerate(budgets):
                jt_n = max(1, _ceil_div(Bq, 128))
                it_n = max(1, _ceil_div(Bc, 128))
                QW = Bq + 8
                CW = Bc + 8
                oqT = ipool.tile([128, KT, QW], f16, tag=f"oqT{s}")
                ocT = ipool.tile([128, KT, CW], f16, tag=f"ocT{s}")
                oqN = ipool.tile([128, jt_n, H], f16, tag=f"oqN{s}")
                geo.append((Bq, Bc, jt_n, it_n, oqT, ocT, oqN))

            # DMA order: slot-0 wt/oqT split in k-ranges (0, 1:4, 4:8) so the
            # ramp-critical Linear can start after ~360KB; bias rides third
            # (only needed at the first Linear eviction). Everything else
            # merged, one trigger per slab (the Sync trigger stream is serial
            # at ~0.6us per dma_start).
            Bq0, Bc0, jt0, _, oqT0, ocT0, oqN0 = geo[0]
            for k in range(KT):
                nc.sync.dma_start(out=wt[:, k, :], in_=wt_d[:, k, :])
                nc.sync.dma_start(
                    out=oqT0[:, k, :], in_=oqT_d[0, :, k, : Bq0 + 8]
                )
                if k == 0:
                    nc.sync.dma_start(out=biasP, in_=bias_d[:, :])
                if k >= 2:
                    # slot-0 score needs ocT right after Linear; interleave
                    # per-k slabs into the ramp stream (the Linear absorbs
                    # the extra trigger latency on the wt/oqT side)
                    ko = k - 2
                    nc.sync.dma_start(
                        out=ocT0[:, ko, :], in_=ocT_d[0, :, ko, : Bc0 + 8]
                    )
            for ko in (6, 7):
                nc.sync.dma_start(
                    out=ocT0[:, ko, :], in_=ocT_d[0, :, ko, : Bc0 + 8]
                )
            nc.sync.dma_start(out=oqN0, in_=oqN_d[0, :, :jt0, :])

            def issue_inputs(s):
                """Input DMAs for slot s, split in k-halves so no single
                trigger blocks the Sync queue for more than ~2.5us
                (descriptor generation scales with bytes)."""
                Bq, Bc, jt_n, it_n, oqT, ocT, oqN = geo[s]
                for lo, hi in ((0, 4), (4, 8)):
                    nc.sync.dma_start(
                        out=oqT[:, lo:hi, :], in_=oqT_d[s, :, lo:hi, : Bq + 8]
                    )
                for lo, hi in ((0, 4), (4, 8)):
                    nc.sync.dma_start(
                        out=ocT[:, lo:hi, :], in_=ocT_d[s, :, lo:hi, : Bc + 8]
                    )
                nc.sync.dma_start(out=oqN, in_=oqN_d[s, :, :jt_n, :])

            # slot 1's inputs go out up front; slot s+2's are issued after
            # slot s's output triggers (see the bottom of the slot loop) so
            # bulky input descriptor-generation never sits ahead of the
            # eviction-critical output triggers in the serial Sync queue.
            if N_SLOTS > 1:
                issue_inputs(1)

            for s in range(N_SLOTS):
                Bq, Bc, jt_n, it_n, oqT, ocT, oqN = geo[s]

                qb = oqT[:, KT - 1, Bq : Bq + jt_n]
                ones = ocT[:, 0, Bc : Bc + 1]

                # ---- Linear: u[o, j] = W @ o_q.T + b ----
                u = wpool.tile([128, OT, Bq], f16, tag=f"u{s}")
                if s == 0:
                    # k-outer with 8 open PSUM o-groups (banks borrowed from
                    # every pool -- nothing else is in PSUM yet): each wt/oqT
                    # k-slab is consumed the moment its DMA lands, so the
                    # ramp is gated by the DMA trigger stream, not by
                    # o-group serialization.
                    pools8 = [ps_u, ps_u, ps_s, ps_s, ps_c, ps_c, ps_c, ps_c]
                    tags8 = ["ups", "ups", "sps", "sps", "cps", "cps", "cps", "cps"]
                    upss = [
                        pools8[o].tile(
                            [128, Bq], f32, tag=tags8[o], name=f"ups0_{o}"
                        )
                        for o in range(OT)
                    ]
                    for k in range(KT):
                        for o in range(OT):
                            nc.tensor.matmul(
                                upss[o],
                                wt[:, k, o * 128 : (o + 1) * 128],
                                oqT[:, k, :Bq],
                                start=(k == 0),
                                stop=(k == KT - 1),
                            )
                    for o in range(OT):
                        nc.vector.tensor_scalar(
                            out=u[:, o, :],
                            in0=upss[o],
                            scalar1=biasP[:, o : o + 1],
                            scalar2=None,
                            op0=mybir.AluOpType.add,
                        )
                else:
                    for o in range(OT):
                        ups = ps_u.tile([128, Bq], f32, tag="ups")
                        for k in range(KT):
                            nc.tensor.matmul(
                                ups,
                                wt[:, k, o * 128 : (o + 1) * 128],
                                oqT[:, k, :Bq],
                                start=(k == 0),
                                stop=(k == KT - 1),
                            )
                        nc.vector.tensor_scalar(
                            out=u[:, o, :],
                            in0=ups,
                            scalar1=biasP[:, o : o + 1],
                            scalar2=None,
                            op0=mybir.AluOpType.add,
                        )

                # ---- score_T + exp: e[j, i] = exp((u.T @ o_cT)/32 + qb[j]).
                # The e tiles are pre-summed on DVE (esum) so the denominator
                # d[1, i] needs a single partition-reduce matmul instead of
                # jt_n of them. Rows [kj, 128) of esum hold the full-tile
                # partial sums only, which is exactly right: the last tile's
                # missing rows don't exist as tokens.
                e_tiles = []
                e_rows = []
                esum = wpool.tile([128, Bc], f16, tag=f"esum{s}")
                for jt in range(jt_n):
                    mj = min(128, Bq - jt * 128)
                    sps = ps_s.tile([128, Bc], f32, tag="sps")
                    for o in range(OT):
                        nc.tensor.matmul(
                            sps[0:mj, :],
                            u[:, o, jt * 128 : jt * 128 + mj],
                            ocT[:, o, :Bc],
                            start=(o == 0),
                            stop=(o == OT - 1),
                        )
                    e = wpool.tile([128, Bc], f16, tag=f"e{s}_{jt}")
                    nc.scalar.activation(
                        out=e[0:mj, :],
                        in_=sps[0:mj, :],
                        func=mybir.ActivationFunctionType.Exp,
                        bias=qb[0:mj, jt : jt + 1],
                        scale=SCALE,
                    )
                    e_tiles.append(e)
                    e_rows.append(mj)
                    if jt == 1:
                        nc.vector.tensor_tensor(
                            out=esum[0 : e_rows[1], :],
                            in0=e_tiles[0][0 : e_rows[1], :],
                            in1=e_tiles[1][0 : e_rows[1], :],
                            op=mybir.AluOpType.add,
                        )
                        if e_rows[1] < 128:
                            nc.vector.tensor_copy(
                                out=esum[e_rows[1] : 128, :],
                                in_=e_tiles[0][e_rows[1] : 128, :],
                            )
                    elif jt >= 2:
                        nc.vector.tensor_tensor(
                            out=esum[0:mj, :],
                            in0=esum[0:mj, :],
                            in1=e[0:mj, :],
                            op=mybir.AluOpType.add,
                        )
                osb_tiles = {}

                def ctx_group_pair(it, mi):
                    """Both hb halves for one i-tile. Even i-tiles take PSUM
                    from ps_c, odd from ps_u (idle during the ctx phase), so
                    two pairs can be in flight without a 9th bank."""
                    if it not in osb_tiles:
                        osb_tiles[it] = opool.tile(
                            [128, H], f16, tag="osb", name=f"osb{it}_{s}"
                        )
                    cps = [
                        ps_c.tile([128, 512], f32, tag="cps", name=f"cps{it}{hb}_{s}")
                        for hb in range(HB)
                    ]
                    for jt in range(jt_n):
                        kj = e_rows[jt]
                        for hb in range(HB):
                            inst = nc.tensor.matmul(
                                cps[hb][0:mi, :],
                                e_tiles[jt][0:kj, it * 128 : it * 128 + mi],
                                oqN[0:kj, jt, hb * 512 : (hb + 1) * 512],
                                start=(jt == 0),
                                stop=(jt == jt_n - 1),
                            )
                            if hb > 0:
                                inst.ins.ldweights = False
                    return cps

                def ctx_evict(it, hb, mi, cps):
                    osb = osb_tiles[it]
                    nc.vector.tensor_copy(
                        out=osb[0:mi, hb * 512 : (hb + 1) * 512],
                        in_=cps[0:mi, :],
                    )
                    if s == N_SLOTS - 1:
                        # drain path: trigger each half as soon as it is
                        # evicted so the final transfers start ~1.5us earlier
                        nc.sync.dma_start(
                            out=out_d[
                                s, it * 128 : it * 128 + mi, hb * 512 : (hb + 1) * 512
                            ],
                            in_=osb[0:mi, hb * 512 : (hb + 1) * 512],
                        )
                    elif hb == HB - 1:
                        nc.sync.dma_start(
                            out=out_d[s, it * 128 : it * 128 + mi, :],
                            in_=osb[0:mi, :],
                        )

                def mi_of(it):
                    return min(128, Bc - it * 128)

                # two ctx pairs in flight before the d-chain: the d-matmul
                # waits on DVE's esum and the 1/d transposes wait on the dsb
                # copy -- pair 1's matmuls keep the PE fed through both.
                pend = {0: ctx_group_pair(0, mi_of(0))}
                if it_n > 1:
                    pend[1] = ctx_group_pair(1, mi_of(1))
                dps = ps_s.tile([1, Bc], f32, tag="sps", name=f"dps_{s}")
                dsrc = esum if jt_n > 1 else e_tiles[0]
                drows = 128 if jt_n > 1 else e_rows[0]
                nc.tensor.matmul(
                    dps,
                    ones[0:drows, :],
                    dsrc[0:drows, :],
                    start=True,
                    stop=True,
                )
                dsb = wpool.tile([1, Bc], f32, tag=f"dsb{s}")
                nc.vector.tensor_copy(out=dsb, in_=dps)
                nc.sync.dma_start(out=d_d[s, 0:1, :Bc], in_=dsb[0:1, :])

                for it in range(it_n):
                    for hb in range(HB):
                        ctx_evict(it, hb, mi_of(it), pend[it][hb])
                    nxt = it + 2
                    if nxt < it_n:
                        # keep one pair in flight ahead of the evictions
                        pend[nxt] = ctx_group_pair(nxt, mi_of(nxt))

                if s + 2 < N_SLOTS:
                    issue_inputs(s + 2)

    nc.compile()
    return nc


def _host_inputs(o_c, o_q, W, b, q_lengths, slots, budgets):
    """Build the per-core input maps (host-side sharding + re-layout)."""
    NEG16 = np.float16(-60000.0)  # exp(x - 60000) == 0 exactly in fp32
    # wt[p, k, col] = W.T[k*128+p, col] (partition-major slab)
    wt_host = np.zeros((128, KT, WTW), np.float16)
    wt_host[:, :, :H] = W.T.reshape(KT, 128, H).transpose(1, 0, 2)
    bias_host = np.ascontiguousarray(b.reshape(OT, 128).T)  # [128, o_tile] f32
    o_q16 = o_q.astype(np.float16)
    o_c16 = o_c.astype(np.float16)
    in_maps = []
    for c in range(N_CORES):
        oqT = np.zeros((N_SLOTS, 128, KT, QTW), np.float16)
        ocT = np.zeros((N_SLOTS, 128, KT, CTW), np.float16)
        oqN = np.zeros((N_SLOTS, 128, JT, H), np.float16)
        for s in range(N_SLOTS):
            g = slots[s][c]
            Bq, Bc = budgets[s]
            jt_n = max(1, _ceil_div(Bq, 128))
            # oqT[p, k, j] = o_q[j, k*128+p]
            oqT[s, :, :, :Tq] = o_q16[g].T.reshape(KT, 128, Tq).transpose(1, 0, 2)
            ocT[s, :, :, :Tc] = o_c16[g].T.reshape(KT, 128, Tc).transpose(1, 0, 2)
            # oqN[p, j, h] = o_q[j*128+p, h]
            oqN[s] = o_q16[g].reshape(JT, 128, H).transpose(1, 0, 2)
            ocT[s, :, 0, Bc] = 1.0  # ones column for the denominator matmul
            ql = int(q_lengths[g])
            jidx = np.arange(jt_n)[None, :] * 128 + np.arange(128)[:, None]
            oqT[s, :, KT - 1, Bq : Bq + jt_n] = np.where(
                jidx < ql, np.float16(0.0), NEG16
            )
        in_maps.append(
            {"oqT": oqT, "ocT": ocT, "oqN": oqN, "wt": wt_host, "biasP": bias_host}
        )
    return in_maps


def kernel(**inputs) -> np.ndarray:
    o_c = np.asarray(inputs["o_c"], dtype=np.float32)
    o_q = np.asarray(inputs["o_q"], dtype=np.float32)
    W = np.asarray(inputs["W"], dtype=np.float32)
    b = np.asarray(inputs["b"], dtype=np.float32)
    q_lengths = np.asarray(inputs["q_lengths"]).astype(np.int64)
    c_lengths = np.asarray(inputs["c_lengths"]).astype(np.int64)

    from concourse.bass_utils import run_bass_kernel_spmd

    slots, budgets = _plan(q_lengths, c_lengths)
    in_maps = _host_inputs(o_c, o_q, W, b, q_lengths, slots, budgets)
    nc = _build_program(budgets)

    trace = bool(int(os.environ.get("KERNEL_TRACE", "0")))
    res = run_bass_kernel_spmd(
        nc, in_maps, core_ids=list(range(N_CORES)), trace=trace
    )
    if trace:
        kernel.last_results = res

    out = np.zeros((B, Tc, H), dtype=np.float32)
    for c in range(N_CORES):
        dev = res.results[c]["out"]
        dvec = np.asarray(res.results[c]["dout"], dtype=np.float32)
        for s in range(N_SLOTS):
            g = slots[s][c]
            cl = int(c_lengths[g])
            out[g, :cl] = dev[s, :cl].astype(np.float32) / dvec[s, 0, :cl, None]
    return out
